# revision 1
# baseline (speedup 1.0000x reference)
"""Conv4d (3,3,3,3) kernel for Trainium2, 8 NeuronCores.

Problem: x (2,24,16,16,48,48) * weight (48,24,3,3,3,3) + bias3d.sum(0)
      -> out (2,48,14,14,46,46), stride 1, no padding.

Strategy
--------
Sharding: 8 cores = (batch 2) x (ol-block 2) x (od-block 2). Each core owns a
7x7 block of (ol, od) output planes (49 tasks).

Per task: implicit GEMM. Contraction rows = (lo, do, ci) = 216 (+1 bias row),
packed on the host into xs[t, 217, 48, 48] where row r = (lo*3+do)*24+ci is
the input plane x[b, ci, ol+lo, od+do, :, :]; row 216 is all-ones. For each
of the 9 (ho, wo) kernel offsets the moving operand is the SBUF-resident
tile sliced [k, oh0+ho : oh0+ho+rows, wo : wo+46]; all offsets accumulate
into a PSUM tile. Bias is weight row 216 (offset (0,0) only) vs the ones row.

Col-pair concurrency: the 128x128 PE array runs two M=48 matmuls
CONCURRENTLY when their outputs sit on disjoint 32-col strips
(tile_position (0,0) and (0,64)); microbenchmarked 98.4 ns/MM vs 197.5
sequential (2.01x) at N=460 fp16. So tasks are processed in pairs: task A
accumulates into PSUM partitions 0:48, task B into 64:112, matmuls emitted
interleaved A,B. Effective PE time ~ half the single-stream 18-pass cost.
The odd leftover task pairs its own output CHUNKS across the two strips.

Input loads split across both HWDGE queues (k1 on SP/sync, k2 on
Activation); all loads are 128-partition DMAs (xs rows padded to 256) so
each InstDMACopy spreads over all 16 SDMA engines. Output staged per task
in SBUF as bf16 and stored with one HWDGE DMA per task.

Measured (NTFF profile, single exec, core 0): 431343 ns vs 1006399 ns for
the sequential single-stream baseline (2.33x); tensor engine ~93% busy, DMA
fully overlapped, first matmul round at ~10 us (all four pair-0 k-loads
split at h=24, critical halves queued first on both HWDGE queues).
"""

import os
import sys

if "/opt/trn_rl_repo" not in sys.path:
    sys.path.insert(0, "/opt/trn_rl_repo")

import numpy as np

from concourse import bacc, bass, tile
from concourse.bass_utils import run_bass_kernel_spmd

mybir = bass.mybir

B, CI, CO = 2, 24, 48
L, D, H, W = 16, 16, 48, 48
OL, OD, OH, OW = 14, 14, 46, 46
N_TASKS = 49  # 7x7 (ol, od) planes per core
KROWS = 217  # (lo,do,ci) contraction rows + ones row
KSPLIT = 128  # k1 = rows 0:128, k2 = rows 128:217

CHUNK_ROWS = (10, 10, 10, 10, 6)
CHUNK_OH0 = (0, 10, 20, 30, 40)

DTYPE = mybir.dt.float16
ODTYPE = mybir.dt.bfloat16
X_BUFS = int(os.environ.get("CONV_XBUFS", "4"))
PS_BUFS = int(os.environ.get("CONV_PSBUFS", "8"))
O_BUFS = int(os.environ.get("CONV_OBUFS", "3"))


def build_program(n_tasks: int = N_TASKS, repeat: int = 1):
    from contextlib import nullcontext

    nc = bacc.Bacc()
    f32 = mybir.dt.float32
    k2rows = KROWS - KSPLIT  # 89

    # xs rows padded 217 -> 256 so BOTH k-tile loads are 128-partition DMAs:
    # a 128-partition InstDMACopy splits 8 descriptors to each of the 16 SDMA
    # engines, while an 89-partition one serializes on a single engine
    # (measured 353 ns/packet vs 175 ns 16-wide).
    xs_d = nc.dram_tensor("xs", [n_tasks, 256, H, W], DTYPE, kind="ExternalInput")
    out_d = nc.dram_tensor("out", [n_tasks, CO, OH, OW], ODTYPE, kind="ExternalOutput")
    w1_d = nc.dram_tensor("w1", [KSPLIT, 9, CO], DTYPE, kind="ExternalInput")
    w2_d = nc.dram_tensor("w2", [k2rows, 9, CO], DTYPE, kind="ExternalInput")

    with tile.TileContext(nc) as tc:
        with (
            tc.tile_pool(name="wpool", bufs=1) as wpool,
            tc.tile_pool(name="xpool", bufs=X_BUFS) as xpool,
            tc.tile_pool(name="opool", bufs=O_BUFS) as opool,
            tc.tile_pool(name="pspool", bufs=PS_BUFS, space="PSUM") as pspool,
            tc.For_i(0, repeat, 1) if repeat > 1 else nullcontext(),
        ):
            w1s = wpool.tile([KSPLIT, 9, CO], DTYPE)
            w2s = wpool.tile([k2rows, 9, CO], DTYPE)
            nc.sync.dma_start(out=w1s[:], in_=w1_d[:])
            nc.scalar.dma_start(out=w2s[:], in_=w2_d[:])

            def emit_mm_pair(ps, wA, rhsA, wB, rhsB, first, last, n):
                nc.tensor.matmul(
                    ps[0:CO, 0:n], lhsT=wA, rhs=rhsA, start=first, stop=last
                )
                if rhsB is not None:
                    nc.tensor.matmul(
                        ps[64 : 64 + CO, 0:n],
                        lhsT=wB,
                        rhs=rhsB,
                        start=first,
                        stop=last,
                    )

            npairs = (n_tasks + 1) // 2
            for p in range(npairs):
                tA = 2 * p
                tB = 2 * p + 1
                has_b = tB < n_tasks

                k1A = xpool.tile([KSPLIT, H, W], DTYPE, tag="k1A")
                k2A = xpool.tile([KSPLIT, H, W], DTYPE, tag="k2A")
                if p == 0:
                    # First pair: land BOTH tasks' first-chunk rows (h < 24)
                    # before any load tails, so round 0's A and B matmuls
                    # both start ~half a load earlier.
                    k1B = xpool.tile([KSPLIT, H, W], DTYPE, tag="k1B")
                    k2B = xpool.tile([KSPLIT, H, W], DTYPE, tag="k2B")
                    nc.sync.dma_start(out=k1A[:, 0:24, :], in_=xs_d[tA, 0:KSPLIT, 0:24, :])
                    nc.scalar.dma_start(
                        out=k2A[:, 0:24, :],
                        in_=xs_d[tA, KSPLIT : KSPLIT + 128, 0:24, :],
                    )
                    nc.sync.dma_start(out=k1B[:, 0:24, :], in_=xs_d[tB, 0:KSPLIT, 0:24, :])
                    nc.scalar.dma_start(
                        out=k2B[:, 0:24, :],
                        in_=xs_d[tB, KSPLIT : KSPLIT + 128, 0:24, :],
                    )
                    nc.sync.dma_start(
                        out=k1A[:, 24:48, :], in_=xs_d[tA, 0:KSPLIT, 24:48, :]
                    )
                    nc.scalar.dma_start(
                        out=k2A[:, 24:48, :],
                        in_=xs_d[tA, KSPLIT : KSPLIT + 128, 24:48, :],
                    )
                    nc.sync.dma_start(
                        out=k1B[:, 24:48, :], in_=xs_d[tB, 0:KSPLIT, 24:48, :]
                    )
                    nc.scalar.dma_start(
                        out=k2B[:, 24:48, :],
                        in_=xs_d[tB, KSPLIT : KSPLIT + 128, 24:48, :],
                    )
                else:
                    nc.sync.dma_start(out=k1A[:], in_=xs_d[tA, 0:KSPLIT])
                    nc.scalar.dma_start(
                        out=k2A[:], in_=xs_d[tA, KSPLIT : KSPLIT + 128]
                    )
                    if has_b:
                        k1B = xpool.tile([KSPLIT, H, W], DTYPE, tag="k1B")
                        k2B = xpool.tile([KSPLIT, H, W], DTYPE, tag="k2B")
                        nc.sync.dma_start(out=k1B[:], in_=xs_d[tB, 0:KSPLIT])
                        nc.scalar.dma_start(
                            out=k2B[:], in_=xs_d[tB, KSPLIT : KSPLIT + 128]
                        )

                oT = opool.tile([112, OH, OW], ODTYPE, tag="oT")

                if has_b:
                    # two tasks run concurrently on PE col strips 0-1 / 2-3
                    for c in range(len(CHUNK_ROWS)):
                        rows = CHUNK_ROWS[c]
                        oh0 = CHUNK_OH0[c]
                        n = rows * OW
                        ps = pspool.tile([112, 512], f32, tag="ps")
                        for idx in range(9):
                            ho, wo = divmod(idx, 3)
                            for kt, (ksA, ksB, ws, krows) in enumerate(
                                ((k1A, k1B, w1s, KSPLIT), (k2A, k2B, w2s, k2rows))
                            ):
                                sl = (
                                    slice(0, krows),
                                    slice(oh0 + ho, oh0 + ho + rows),
                                    slice(wo, wo + OW),
                                )
                                emit_mm_pair(
                                    ps,
                                    ws[:, idx, :],
                                    ksA[sl],
                                    ws[:, idx, :],
                                    ksB[sl],
                                    idx == 0 and kt == 0,
                                    idx == 8 and kt == 1,
                                    n,
                                )
                        nc.vector.tensor_copy(
                            out=oT[0:112, oh0 : oh0 + rows, :], in_=ps[0:112, 0:n]
                        )
                    nc.sync.dma_start(out=out_d[tA], in_=oT[0:CO])
                    nc.scalar.dma_start(out=out_d[tB], in_=oT[64 : 64 + CO])
                else:
                    # leftover task: pair its own CHUNKS on the two col strips
                    for cA in range(0, len(CHUNK_ROWS), 2):
                        cB = cA + 1 if cA + 1 < len(CHUNK_ROWS) else None
                        rowsA, ohA = CHUNK_ROWS[cA], CHUNK_OH0[cA]
                        nA = rowsA * OW
                        ps = pspool.tile([112, 512], f32, tag="ps")
                        for idx in range(9):
                            ho, wo = divmod(idx, 3)
                            for kt, (ks, ws, krows) in enumerate(
                                ((k1A, w1s, KSPLIT), (k2A, w2s, k2rows))
                            ):
                                first = idx == 0 and kt == 0
                                last = idx == 8 and kt == 1
                                nc.tensor.matmul(
                                    ps[0:CO, 0:nA],
                                    lhsT=ws[:, idx, :],
                                    rhs=ks[
                                        0:krows,
                                        ohA + ho : ohA + ho + rowsA,
                                        wo : wo + OW,
                                    ],
                                    start=first,
                                    stop=last,
                                )
                                if cB is not None:
                                    rowsB, ohB = CHUNK_ROWS[cB], CHUNK_OH0[cB]
                                    nc.tensor.matmul(
                                        ps[64 : 64 + CO, 0 : rowsB * OW],
                                        lhsT=ws[:, idx, :],
                                        rhs=ks[
                                            0:krows,
                                            ohB + ho : ohB + ho + rowsB,
                                            wo : wo + OW,
                                        ],
                                        start=first,
                                        stop=last,
                                    )
                        nc.vector.tensor_copy(
                            out=oT[0:CO, ohA : ohA + rowsA, :], in_=ps[0:CO, 0:nA]
                        )
                        nc.sync.dma_start(
                            out=out_d[tA, :, ohA : ohA + rowsA], in_=oT[0:CO, ohA : ohA + rowsA]
                        )
                        if cB is not None:
                            rowsB, ohB = CHUNK_ROWS[cB], CHUNK_OH0[cB]
                            nc.vector.tensor_copy(
                                out=oT[64 : 64 + CO, ohB : ohB + rowsB, :],
                                in_=ps[64 : 64 + CO, 0 : rowsB * OW],
                            )
                            nc.scalar.dma_start(
                                out=out_d[tA, :, ohB : ohB + rowsB],
                                in_=oT[64 : 64 + CO, ohB : ohB + rowsB],
                            )
    nc.finalize()
    return nc


def make_in_maps(x, weight, bias3d, n_tasks: int = N_TASKS):
    """Host-side shard + repack into the per-task packed-row layout."""
    npdt = mybir.dt.np(DTYPE)
    x = np.asarray(x, np.float32)
    weight = np.asarray(weight, np.float32)
    bias3d = np.asarray(bias3d, np.float32)

    # W[(lo*3+do)*24+ci, ho*3+wo, co] = weight[co, ci, lo, do, ho, wo]
    Wr = np.ascontiguousarray(np.transpose(weight, (2, 3, 1, 4, 5, 0))).reshape(
        216, 9, CO
    )
    Wfull = np.zeros((KROWS, 9, CO), np.float32)
    Wfull[:216] = Wr
    Wfull[216, 0, :] = bias3d.sum(axis=0)
    w1 = np.ascontiguousarray(Wfull[:KSPLIT]).astype(npdt)
    w2 = np.ascontiguousarray(Wfull[KSPLIT:]).astype(npdt)

    in_maps = []
    for c in range(8):
        b, lb, db = c // 4, (c // 2) % 2, c % 2
        slab = np.ascontiguousarray(
            x[b, :, 7 * lb : 7 * lb + 9, 7 * db : 7 * db + 9]
        )  # (24, 9, 9, 48, 48)
        s_ci, s_l, s_d, s_h, s_w = slab.strides
        # V[l0, d0, lo, do, ci, h, w] = slab[ci, l0+lo, d0+do, h, w]
        V = np.lib.stride_tricks.as_strided(
            slab,
            shape=(7, 7, 3, 3, CI, H, W),
            strides=(s_l, s_d, s_l, s_d, s_ci, s_h, s_w),
        )
        xs = np.zeros((N_TASKS, 256, H, W), np.float32)
        xs[:, :216] = V.reshape(N_TASKS, 216, H, W)
        xs[:, 216] = 1.0
        in_maps.append({"xs": xs[:n_tasks].astype(npdt), "w1": w1, "w2": w2})
    return in_maps


def assemble_output(results):
    out = np.empty((B, CO, OL, OD, OH, OW), np.float32)
    for c in range(8):
        b, lb, db = c // 4, (c // 2) % 2, c % 2
        r = np.asarray(results[c]["out"]).astype(np.float32).reshape(7, 7, CO, OH, OW)
        out[b, :, 7 * lb : 7 * lb + 7, 7 * db : 7 * db + 7] = r.transpose(2, 0, 1, 3, 4)
    return out


_NC_CACHE = {}


def _get_program():
    if "nc" not in _NC_CACHE:
        _NC_CACHE["nc"] = build_program()
    return _NC_CACHE["nc"]


def kernel(x, weight, bias3d):
    nc = _get_program()
    in_maps = make_in_maps(x, weight, bias3d)
    res = run_bass_kernel_spmd(nc, in_maps, list(range(8))).results
    return assemble_output(res)



# revision 13
# speedup vs baseline: 1.0655x; 1.0655x over previous
"""Conv4d (3,3,3,3) kernel for Trainium2, 8 NeuronCores.

Problem: x (2,24,16,16,48,48) * weight (48,24,3,3,3,3) + bias3d.sum(0)
      -> out (2,48,14,14,46,46), stride 1, no padding.

Strategy
--------
Sharding: 8 cores = (batch 2) x (ol-block 2) x (od-block 2). Each core owns a
7x7 block of (ol, od) output planes (49 tasks).

Per task: implicit GEMM. Contraction rows = (lo, do, ci) = 216 (+1 bias row),
packed on the host into xs[t, 217, 48, 48] where row r = (lo*3+do)*24+ci is
the input plane x[b, ci, ol+lo, od+do, :, :]; row 216 is all-ones. For each
of the 9 (ho, wo) kernel offsets the moving operand is the SBUF-resident
tile sliced [k, oh0+ho : oh0+ho+rows, wo : wo+46]; all offsets accumulate
into a PSUM tile. Bias is weight row 216 (offset (0,0) only) vs the ones row.

Col-pair concurrency: the 128x128 PE array runs two M=48 matmuls
CONCURRENTLY when their outputs sit on disjoint 32-col strips
(tile_position (0,0) and (0,64)); microbenchmarked 98.4 ns/MM vs 197.5
sequential (2.01x) at N=460 fp16. So tasks are processed in pairs: task A
accumulates into PSUM partitions 0:48, task B into 64:112, matmuls emitted
interleaved A,B. Effective PE time ~ half the single-stream 18-pass cost.
The odd leftover task pairs its own output CHUNKS across the two strips.

Input loads split across both HWDGE queues (k1 on SP/sync, k2 on
Activation); all loads are 128-partition DMAs (xs rows padded to 256) so
each InstDMACopy spreads over all 16 SDMA engines. Output staged per task
in SBUF as bf16 and stored with one HWDGE DMA per task.

Measured (NTFF profile, single exec, core 0): 431343 ns vs 1006399 ns for
the sequential single-stream baseline (2.33x); tensor engine ~93% busy, DMA
fully overlapped, first matmul round at ~10 us (all four pair-0 k-loads
split at h=24, critical halves queued first on both HWDGE queues).
"""

import os
import sys

if "/opt/trn_rl_repo" not in sys.path:
    sys.path.insert(0, "/opt/trn_rl_repo")

import numpy as np

from concourse import bacc, bass, tile
from concourse.bass_utils import run_bass_kernel_spmd

mybir = bass.mybir

B, CI, CO = 2, 24, 48
L, D, H, W = 16, 16, 48, 48
OL, OD, OH, OW = 14, 14, 46, 46
N_TASKS = 49  # 7x7 (ol, od) planes per core
KROWS = 217  # (lo,do,ci) contraction rows + ones row
KSPLIT = 128  # k1 = rows 0:128, k2 = rows 128:217

CHUNK_ROWS = (10, 10, 10, 10, 6)
CHUNK_OH0 = (0, 10, 20, 30, 40)

DTYPE = mybir.dt.float16
ODTYPE = mybir.dt.bfloat16
X_BUFS = int(os.environ.get("CONV_XBUFS", "4"))
PS_BUFS = int(os.environ.get("CONV_PSBUFS", "8"))
O_BUFS = int(os.environ.get("CONV_OBUFS", "3"))
N_WARM = int(os.environ.get("CONV_WARM", "64"))


def build_program(n_tasks: int = N_TASKS, repeat: int = 1):
    from contextlib import nullcontext

    nc = bacc.Bacc()
    f32 = mybir.dt.float32
    k2rows = KROWS - KSPLIT  # 89
    warm = N_WARM if repeat == 1 else 0
    ps_bufs = min(PS_BUFS, 7) if warm else PS_BUFS  # warmup takes the 8th bank

    # xs rows padded 217 -> 256 so BOTH k-tile loads are 128-partition DMAs:
    # a 128-partition InstDMACopy splits 8 descriptors to each of the 16 SDMA
    # engines, while an 89-partition one serializes on a single engine
    # (measured 353 ns/packet vs 175 ns 16-wide).
    xs_d = nc.dram_tensor("xs", [n_tasks, 256, H, W], DTYPE, kind="ExternalInput")
    out_d = nc.dram_tensor("out", [n_tasks, CO, OH, OW], ODTYPE, kind="ExternalOutput")
    w1_d = nc.dram_tensor("w1", [KSPLIT, 9, CO], DTYPE, kind="ExternalInput")
    w2_d = nc.dram_tensor("w2", [k2rows, 9, CO], DTYPE, kind="ExternalInput")

    with tile.TileContext(nc) as tc:
        with (
            tc.tile_pool(name="wpool", bufs=1) as wpool,
            tc.tile_pool(name="xpool", bufs=X_BUFS) as xpool,
            tc.tile_pool(name="opool", bufs=O_BUFS) as opool,
            tc.tile_pool(name="pspool", bufs=ps_bufs, space="PSUM") as pspool,
            (
                tc.tile_pool(name="wmpool", bufs=1, space="PSUM")
                if warm
                else nullcontext()
            ) as wmpool,
            tc.For_i(0, repeat, 1) if repeat > 1 else nullcontext(),
        ):
            w1s = wpool.tile([KSPLIT, 9, CO], DTYPE)
            w2s = wpool.tile([k2rows, 9, CO], DTYPE)
            nc.sync.dma_start(out=w1s[:], in_=w1_d[:])
            nc.scalar.dma_start(out=w2s[:], in_=w2_d[:])

            if warm:
                # HAM warmup: dependency-free dummy matmuls on uninitialized
                # SBUF keep the PE busy during the ~6us initial k-load DMA so
                # the first real matmuls run at K=8/8 (2.4 GHz) instead of
                # paying the ~3.4us cold window at 1.2 GHz. Never read back.
                dmy = wpool.tile([128, 128], DTYPE, tag="warm_in")
                wps = wmpool.tile([128, 128], mybir.dt.float32, tag="warm_ps")
                nc.vector.memzero(dmy[:])
                for _ in range(warm):
                    nc.tensor.matmul(
                        wps[:], lhsT=dmy[:], rhs=dmy[:], start=True, stop=True
                    )

            def emit_mm_pair(ps, wA, rhsA, wB, rhsB, first, last, n):
                nc.tensor.matmul(
                    ps[0:CO, 0:n], lhsT=wA, rhs=rhsA, start=first, stop=last
                )
                if rhsB is not None:
                    nc.tensor.matmul(
                        ps[64 : 64 + CO, 0:n],
                        lhsT=wB,
                        rhs=rhsB,
                        start=first,
                        stop=last,
                    )

            npairs = (n_tasks + 1) // 2
            for p in range(npairs):
                tA = 2 * p
                tB = 2 * p + 1
                has_b = tB < n_tasks

                k1A = xpool.tile([KSPLIT, H, W], DTYPE, tag="k1A")
                k2A = xpool.tile([KSPLIT, H, W], DTYPE, tag="k2A")
                if p == 0:
                    # First pair: land BOTH tasks' chunk-0-critical rows
                    # (h < 12) first, then the rest in two waves, so round 0's
                    # A and B matmuls start as early as the DMA allows.
                    k1B = xpool.tile([KSPLIT, H, W], DTYPE, tag="k1B")
                    k2B = xpool.tile([KSPLIT, H, W], DTYPE, tag="k2B")
                    for h0, h1 in ((0, 12), (12, 24), (24, 48)):
                        nc.sync.dma_start(
                            out=k1A[:, h0:h1, :], in_=xs_d[tA, 0:KSPLIT, h0:h1, :]
                        )
                        nc.scalar.dma_start(
                            out=k2A[:, h0:h1, :],
                            in_=xs_d[tA, KSPLIT : KSPLIT + 128, h0:h1, :],
                        )
                        nc.sync.dma_start(
                            out=k1B[:, h0:h1, :], in_=xs_d[tB, 0:KSPLIT, h0:h1, :]
                        )
                        nc.scalar.dma_start(
                            out=k2B[:, h0:h1, :],
                            in_=xs_d[tB, KSPLIT : KSPLIT + 128, h0:h1, :],
                        )
                else:
                    nc.sync.dma_start(out=k1A[:], in_=xs_d[tA, 0:KSPLIT])
                    nc.scalar.dma_start(
                        out=k2A[:], in_=xs_d[tA, KSPLIT : KSPLIT + 128]
                    )
                    if has_b:
                        k1B = xpool.tile([KSPLIT, H, W], DTYPE, tag="k1B")
                        k2B = xpool.tile([KSPLIT, H, W], DTYPE, tag="k2B")
                        nc.sync.dma_start(out=k1B[:], in_=xs_d[tB, 0:KSPLIT])
                        nc.scalar.dma_start(
                            out=k2B[:], in_=xs_d[tB, KSPLIT : KSPLIT + 128]
                        )

                oT = opool.tile([112, OH, OW], ODTYPE, tag="oT")

                if has_b:
                    # two tasks run concurrently on PE col strips 0-1 / 2-3
                    for c in range(len(CHUNK_ROWS)):
                        rows = CHUNK_ROWS[c]
                        oh0 = CHUNK_OH0[c]
                        n = rows * OW
                        ps = pspool.tile([112, 512], f32, tag="ps")
                        for idx in range(9):
                            ho, wo = divmod(idx, 3)
                            for kt, (ksA, ksB, ws, krows) in enumerate(
                                ((k1A, k1B, w1s, KSPLIT), (k2A, k2B, w2s, k2rows))
                            ):
                                sl = (
                                    slice(0, krows),
                                    slice(oh0 + ho, oh0 + ho + rows),
                                    slice(wo, wo + OW),
                                )
                                emit_mm_pair(
                                    ps,
                                    ws[:, idx, :],
                                    ksA[sl],
                                    ws[:, idx, :],
                                    ksB[sl],
                                    idx == 0 and kt == 0,
                                    idx == 8 and kt == 1,
                                    n,
                                )
                        nc.vector.tensor_copy(
                            out=oT[0:112, oh0 : oh0 + rows, :], in_=ps[0:112, 0:n]
                        )
                    nc.sync.dma_start(out=out_d[tA], in_=oT[0:CO])
                    nc.scalar.dma_start(out=out_d[tB], in_=oT[64 : 64 + CO])
                else:
                    # leftover task: pair its own CHUNKS on the two col strips
                    for cA in range(0, len(CHUNK_ROWS), 2):
                        cB = cA + 1 if cA + 1 < len(CHUNK_ROWS) else None
                        rowsA, ohA = CHUNK_ROWS[cA], CHUNK_OH0[cA]
                        nA = rowsA * OW
                        ps = pspool.tile([112, 512], f32, tag="ps")
                        for idx in range(9):
                            ho, wo = divmod(idx, 3)
                            for kt, (ks, ws, krows) in enumerate(
                                ((k1A, w1s, KSPLIT), (k2A, w2s, k2rows))
                            ):
                                first = idx == 0 and kt == 0
                                last = idx == 8 and kt == 1
                                nc.tensor.matmul(
                                    ps[0:CO, 0:nA],
                                    lhsT=ws[:, idx, :],
                                    rhs=ks[
                                        0:krows,
                                        ohA + ho : ohA + ho + rowsA,
                                        wo : wo + OW,
                                    ],
                                    start=first,
                                    stop=last,
                                )
                                if cB is not None:
                                    rowsB, ohB = CHUNK_ROWS[cB], CHUNK_OH0[cB]
                                    nc.tensor.matmul(
                                        ps[64 : 64 + CO, 0 : rowsB * OW],
                                        lhsT=ws[:, idx, :],
                                        rhs=ks[
                                            0:krows,
                                            ohB + ho : ohB + ho + rowsB,
                                            wo : wo + OW,
                                        ],
                                        start=first,
                                        stop=last,
                                    )
                        nc.vector.tensor_copy(
                            out=oT[0:CO, ohA : ohA + rowsA, :], in_=ps[0:CO, 0:nA]
                        )
                        nc.sync.dma_start(
                            out=out_d[tA, :, ohA : ohA + rowsA], in_=oT[0:CO, ohA : ohA + rowsA]
                        )
                        if cB is not None:
                            rowsB, ohB = CHUNK_ROWS[cB], CHUNK_OH0[cB]
                            nc.vector.tensor_copy(
                                out=oT[64 : 64 + CO, ohB : ohB + rowsB, :],
                                in_=ps[64 : 64 + CO, 0 : rowsB * OW],
                            )
                            nc.scalar.dma_start(
                                out=out_d[tA, :, ohB : ohB + rowsB],
                                in_=oT[64 : 64 + CO, ohB : ohB + rowsB],
                            )
    nc.finalize()
    return nc


def make_in_maps(x, weight, bias3d, n_tasks: int = N_TASKS):
    """Host-side shard + repack into the per-task packed-row layout."""
    npdt = mybir.dt.np(DTYPE)
    x = np.asarray(x, np.float32)
    weight = np.asarray(weight, np.float32)
    bias3d = np.asarray(bias3d, np.float32)

    # W[(lo*3+do)*24+ci, ho*3+wo, co] = weight[co, ci, lo, do, ho, wo]
    Wr = np.ascontiguousarray(np.transpose(weight, (2, 3, 1, 4, 5, 0))).reshape(
        216, 9, CO
    )
    Wfull = np.zeros((KROWS, 9, CO), np.float32)
    Wfull[:216] = Wr
    Wfull[216, 0, :] = bias3d.sum(axis=0)
    w1 = np.ascontiguousarray(Wfull[:KSPLIT]).astype(npdt)
    w2 = np.ascontiguousarray(Wfull[KSPLIT:]).astype(npdt)

    in_maps = []
    for c in range(8):
        b, lb, db = c // 4, (c // 2) % 2, c % 2
        slab = np.ascontiguousarray(
            x[b, :, 7 * lb : 7 * lb + 9, 7 * db : 7 * db + 9]
        )  # (24, 9, 9, 48, 48)
        s_ci, s_l, s_d, s_h, s_w = slab.strides
        # V[l0, d0, lo, do, ci, h, w] = slab[ci, l0+lo, d0+do, h, w]
        V = np.lib.stride_tricks.as_strided(
            slab,
            shape=(7, 7, 3, 3, CI, H, W),
            strides=(s_l, s_d, s_l, s_d, s_ci, s_h, s_w),
        )
        xs = np.zeros((N_TASKS, 256, H, W), np.float32)
        xs[:, :216] = V.reshape(N_TASKS, 216, H, W)
        xs[:, 216] = 1.0
        in_maps.append({"xs": xs[:n_tasks].astype(npdt), "w1": w1, "w2": w2})
    return in_maps


def assemble_output(results):
    out = np.empty((B, CO, OL, OD, OH, OW), np.float32)
    for c in range(8):
        b, lb, db = c // 4, (c // 2) % 2, c % 2
        r = np.asarray(results[c]["out"]).astype(np.float32).reshape(7, 7, CO, OH, OW)
        out[b, :, 7 * lb : 7 * lb + 7, 7 * db : 7 * db + 7] = r.transpose(2, 0, 1, 3, 4)
    return out


_NC_CACHE = {}


def _get_program():
    if "nc" not in _NC_CACHE:
        _NC_CACHE["nc"] = build_program()
    return _NC_CACHE["nc"]


def kernel(x, weight, bias3d):
    nc = _get_program()
    in_maps = make_in_maps(x, weight, bias3d)
    res = run_bass_kernel_spmd(nc, in_maps, list(range(8))).results
    return assemble_output(res)

